# revision 11
# baseline (speedup 1.0000x reference)
"""Causal self-attention (B=4, T=2048, D=1024, H=16) on 8 trn2 NeuronCores.

Sharding: 2-D data x tensor parallel. Core c handles batch b = c//2 and
head group hg = c%2 (8 of the 16 heads). Each core computes its 8 heads'
qkv projection, causal attention, and a partial output projection
(columns of w_out for its heads); the host sums the two partials per
batch element and adds b_out.

v2 pipeline: the attention phase is ACT(exp)-paced (~1.07us/tile) while
the PE needs only ~0.64us/tile -> all projection work (qk/v proj of
later pairs, out proj) is emitted as debt-paced FILLER matmul granules
inside the attention stream so the PE never idles and HAM stays at
K=8/8 (the baseline lost ~40us to K=4/8 oscillation from PE micro-idle).
PSUM budget (8 banks): S tiles [128,1024]f32 ring-2 (4) + per-head PV
[65,512]f32 ring-3 (3) + filler granule [128,512]f32 ring-1 (1).
Head-1's PV matmuls are themselves deferred filler (they only need the
stored exp tiles), which is what frees the granule bank vs the baseline
[65,1024] double-buffered PV.
  - q/k projection -> PSUM f32; DVE add (bias, psum->bf16) keeps ACT free
    for exp. v_aug [tok, 65] bf16 (65th col ones => PSUM row 64 = softmax
    denominator); DVE reciprocal (psum-direct) + GpSimd partition-
    broadcast normalize.
  - S^T tiles = matmul(lhsT=k[64,128], rhs=q[64,512]); two heads packed
    into PE rows 0-63 / 64-127 via tile_position (concurrent streams).
    Diagonal blocks narrowed to the live query range.
  - S(kt+1) emitted before exp(kt)/PV(kt): PE streams the next score
    tile while ACT exponentiates the current one.
"""
import sys

import numpy as np

if "/opt/trn_rl_repo" not in sys.path:
    sys.path.insert(0, "/opt/trn_rl_repo")

import ml_dtypes

D = 1024          # d_model
T = 2048          # seq len
B = 4             # batch
HD = 64           # head dim
KT = 8            # d_model k-tiles of 128
NTT = 16          # token tiles of 128
NTB = 4           # token blocks of 512
NPAIR = 4         # head pairs per core (8 heads)
VSTR = 8 * 65     # v_aug cols per token tile (8 heads x 65)
SCALE = 1.0 / np.sqrt(HD)
WARMUP_MM = 16

_CACHE = {}

QB0 = (0, 4, 12, 24)                       # tile index base per qb


def _tidx(p, qb, kt):
    return p * 40 + QB0[qb] + kt


def _build_program():
    import heapq

    import concourse.mybir as mybir
    import concourse.tile as tile
    from concourse import bacc

    dt = mybir.dt
    f32, bf16 = dt.float32, dt.bfloat16
    AF = mybir.ActivationFunctionType

    nc = bacc.Bacc("TRN2", target_bir_lowering=False, debug=False,
                   enable_asserts=False, num_devices=8)

    x16_d = nc.dram_tensor("x16", [128, KT * T], bf16, kind="ExternalInput").ap()
    wqk_d = nc.dram_tensor("wqk16", [128, 8192], bf16, kind="ExternalInput").ap()
    wv16_d = nc.dram_tensor("wv16", [128, KT * 512], bf16, kind="ExternalInput").ap()
    bqk_d = nc.dram_tensor("bqk", [128, 8], f32, kind="ExternalInput").ap()
    bv_d = nc.dram_tensor("bv", [128, 512], f32, kind="ExternalInput").ap()
    woT_d = nc.dram_tensor("woT", [512, 1024], bf16, kind="ExternalInput").ap()
    mask_d = nc.dram_tensor("mask2", [128, 256], bf16, kind="ExternalInput").ap()
    outT_d = nc.dram_tensor("outT", [D, T], bf16, kind="ExternalOutput").ap()
    warm_d = nc.dram_tensor("warm", [1, 512], f32, kind="ExternalOutput").ap()
    import os
    _dbg = os.environ.get("BASS_DEBUG_DUMP") == "1"
    if _dbg:
        dbg_qk_d = nc.dram_tensor("dbg_qk", [128, 8 * T], bf16,
                                  kind="ExternalOutput").ap()
        dbg_v_d = nc.dram_tensor("dbg_v", [128, NTT * VSTR], bf16,
                                 kind="ExternalOutput").ap()
        dbg_at_d = nc.dram_tensor("dbg_at", [128, NPAIR * T], bf16,
                                  kind="ExternalOutput").ap()

    with tile.TileContext(nc) as tc:
        with tc.tile_pool(name="const", bufs=1) as cpool, \
             tc.tile_pool(name="qk16", bufs=1) as qkpool, \
             tc.tile_pool(name="xt", bufs=1) as xpool, \
             tc.tile_pool(name="vt", bufs=1) as vpool, \
             tc.tile_pool(name="exp", bufs=8) as epool, \
             tc.tile_pool(name="at", bufs=1) as apool, \
             tc.tile_pool(name="rcp", bufs=4) as rpool, \
             tc.tile_pool(name="rbc", bufs=4) as rbpool, \
             tc.tile_pool(name="stg", bufs=3) as spool, \
             tc.tile_pool(name="big", bufs=2, space="PSUM") as pp_big, \
             tc.tile_pool(name="pv", bufs=3, space="PSUM") as pp_pv, \
             tc.tile_pool(name="pj", bufs=1, space="PSUM") as pp_pj:

            # ---- exp table preload (so first real exp pays no load) ----
            dum_i = cpool.tile([128, 8], f32, tag="dmi")
            nc.vector.memset(dum_i[:], 0.0)
            dum_o = cpool.tile([128, 8], bf16, tag="dmo")
            nc.scalar.activation(dum_o[:], dum_i[:], AF.Exp, scale=1.0)

            # ---- PE warm-up: keep the clock un-throttled during DMA ----
            wtile = cpool.tile([128, 512], bf16, tag="wrm")
            nc.vector.memset(wtile[:], 0.001)
            wps = pp_big.tile([128, 1024], f32, tag="big")
            for i in range(WARMUP_MM):
                nc.tensor.matmul(wps[:, 0:512], wtile[:, 0:128], wtile[:],
                                 start=(i == 0), stop=(i == WARMUP_MM - 1))
            wout = cpool.tile([1, 512], f32, tag="wout")
            nc.vector.tensor_copy(wout[:], wps[0:1, 0:512])
            nc.sync.dma_start(warm_d, wout[:])

            # ---- constant loads (ordered: first-needed first) ----
            bqk_sb = cpool.tile([128, 8], f32, tag="bqk")
            nc.sync.dma_start(bqk_sb[:], bqk_d)
            mask_sb = cpool.tile([128, 256], bf16, tag="mask")
            nc.sync.dma_start(mask_sb[:], mask_d)
            wqk_sb = cpool.tile([128, 8192], bf16, tag="wqk")
            for m in (0, 4):
                nc.sync.dma_start(wqk_sb[:, m * 1024:(m + 1) * 1024],
                                  wqk_d[:, m * 1024:(m + 1) * 1024])
            x16_sb = xpool.tile([128, KT * T], bf16, tag="x16")
            for half in range(2):
                for kt in range(KT):
                    nc.sync.dma_start(
                        x16_sb[:, kt * T + half * 1024: kt * T + (half + 1) * 1024],
                        x16_d[:, kt * T + half * 1024: kt * T + (half + 1) * 1024])
                if half == 0:
                    wv16_sb = cpool.tile([128, KT * 512], bf16, tag="wv")
                    nc.sync.dma_start(wv16_sb[:], wv16_d)
                    bv_sb = cpool.tile([128, 512], f32, tag="bv")
                    nc.sync.dma_start(bv_sb[:], bv_d)
            for m in (1, 5):
                nc.sync.dma_start(wqk_sb[:, m * 1024:(m + 1) * 1024],
                                  wqk_d[:, m * 1024:(m + 1) * 1024])
            woT_sb = cpool.tile([128, 4 * 1024], bf16, tag="wo")
            nc.sync.dma_start(
                woT_sb[:].rearrange("p (k f) -> p k f", k=4),
                woT_d.rearrange("(k p) f -> p k f", p=128))
            for m in (2, 6, 3, 7):
                nc.sync.dma_start(wqk_sb[:, m * 1024:(m + 1) * 1024],
                                  wqk_d[:, m * 1024:(m + 1) * 1024])

            # ---- persistent SBUF tiles ----
            qk16_sb = qkpool.tile([128, 8 * T], bf16, tag="qk16")
            v_sb = vpool.tile([128, NTT * VSTR], bf16, tag="v")
            nc.vector.memset(
                v_sb[:].rearrange("p (g e) -> p g e", e=65)[:, :, 64:65], 1.0)
            attn_sb = apool.tile([128, NPAIR * T], bf16, tag="attn")

            # ---- head-phase chunk emitters ([128,1024] psum, tag big) ----
            def qk_chunk(m, nb2):
                ps = pp_big.tile([128, 1024], f32, tag="big")
                for kt in range(KT):
                    for u in range(2):
                        c0 = nb2 * 1024 + u * 512
                        nc.tensor.matmul(
                            ps[:, u * 512:(u + 1) * 512],
                            wqk_sb[:, m * 1024 + kt * 128:
                                   m * 1024 + (kt + 1) * 128],
                            x16_sb[:, kt * T + c0: kt * T + c0 + 512],
                            start=(kt == 0), stop=(kt == KT - 1))
                nc.vector.tensor_add(
                    qk16_sb[:, m * T + nb2 * 1024: m * T + (nb2 + 1) * 1024],
                    ps[:], bqk_sb[:, m:m + 1].to_broadcast([128, 1024]))

            def v_chunk(tt2):
                ps = pp_big.tile([128, 1024], f32, tag="big")
                for kt in range(KT):
                    for u in range(2):
                        tt = 2 * tt2 + u
                        nc.tensor.matmul(
                            ps[:, u * 512:(u + 1) * 512],
                            x16_sb[:, kt * T + tt * 128: kt * T + (tt + 1) * 128],
                            wv16_sb[:, kt * 512:(kt + 1) * 512],
                            start=(kt == 0), stop=(kt == KT - 1))
                nc.vector.tensor_add(
                    v_sb[:].rearrange("p (t h e) -> p t h e", t=NTT, h=8)
                        [:, 2 * tt2:2 * tt2 + 2, :, 0:64],
                    ps[:].rearrange("p (u h f) -> p u h f", u=2, h=8),
                    bv_sb[:].rearrange("p (u h f) -> p u h f", u=1, h=8)
                         .to_broadcast([128, 2, 8, 64]))

            # ---- filler granule emitters ([128,512] psum, tag pj) ----
            pj_state = {}

            def qk_half(m, nb, h):
                if h == 0:
                    pj_state[(m, nb)] = pp_pj.tile([128, 512], f32, tag="pj", name="pjqk")
                ps = pj_state[(m, nb)]
                for kt in range(4 * h, 4 * h + 4):
                    nc.tensor.matmul(
                        ps[:],
                        wqk_sb[:, m * 1024 + kt * 128: m * 1024 + (kt + 1) * 128],
                        x16_sb[:, kt * T + nb * 512: kt * T + (nb + 1) * 512],
                        start=(kt == 0), stop=(kt == KT - 1))
                if h == 1:
                    del pj_state[(m, nb)]
                    nc.vector.tensor_add(
                        qk16_sb[:, m * T + nb * 512: m * T + (nb + 1) * 512],
                        ps[:], bqk_sb[:, m:m + 1].to_broadcast([128, 512]))

            def v_half(tt, h):
                if h == 0:
                    pj_state[("v", tt)] = pp_pj.tile([128, 512], f32, tag="pj", name="pjv")
                ps = pj_state[("v", tt)]
                for kt in range(4 * h, 4 * h + 4):
                    nc.tensor.matmul(
                        ps[:],
                        x16_sb[:, kt * T + tt * 128: kt * T + (tt + 1) * 128],
                        wv16_sb[:, kt * 512:(kt + 1) * 512],
                        start=(kt == 0), stop=(kt == KT - 1))
                if h == 1:
                    del pj_state[("v", tt)]
                    nc.vector.tensor_add(
                        v_sb[:].rearrange("p (t h e) -> p t h e", t=NTT, h=8)
                            [:, tt:tt + 1, :, 0:64],
                        ps[:].rearrange("p (u h f) -> p u h f", u=1, h=8),
                        bv_sb[:].rearrange("p (u h f) -> p u h f", u=1, h=8)
                             .to_broadcast([128, 1, 8, 64]))

            def out_chunk(md, tb2):
                ps = pp_big.tile([128, 1024], f32, tag="big")
                for kf in range(4):
                    for u in range(2):
                        tb = 2 * tb2 + u
                        nc.tensor.matmul(
                            ps[:, u * 512:(u + 1) * 512],
                            woT_sb[:, kf * 1024 + md * 128:
                                   kf * 1024 + (md + 1) * 128],
                            attn_sb[:, kf * T + tb * 512: kf * T + (tb + 1) * 512],
                            start=(kf == 0), stop=(kf == 3))
                st = spool.tile([128, 1024], bf16, tag="stg")
                nc.vector.tensor_copy(st[:], ps[:])
                nc.sync.dma_start(
                    outT_d[md * 128:(md + 1) * 128,
                           tb2 * 1024:(tb2 + 1) * 1024], st[:])

            # ---- normalize chain (per head) ----
            def norm(p, qb, hh, pv_t):
                den = rpool.tile([1, 512], f32, tag="den")
                nc.vector.tensor_copy(den[:], pv_t[64:65, :])
                rc = rpool.tile([1, 512], f32, tag="rc")
                nc.vector.reciprocal_approx_fast(rc[:], den[:])
                rb = rbpool.tile([64, 512], f32, tag="rb")
                nc.gpsimd.partition_broadcast(rb[:], rc[:])
                nc.vector.tensor_mul(
                    attn_sb[hh * 64:(hh + 1) * 64,
                            p * T + qb * 512: p * T + (qb + 1) * 512],
                    pv_t[0:64, :], rb[:])

            # ---- attention stream ----
            seq = [(p, qb, kt)
                   for p in range(NPAIR)
                   for qb in range(NTB)
                   for kt in range(4 * qb + 4)]
            sc_tiles = {}
            et_tiles = {}

            def emit_s(p, qb, kt):
                off = max(kt - 4 * qb, 0) * 128
                sc = pp_big.tile([128, 1024], f32, tag="big")
                for hh in range(2):
                    nc.tensor.matmul(
                        sc[:, hh * 512 + off:(hh + 1) * 512],
                        qk16_sb[(hh * 64):(hh + 1) * 64,
                                (4 + p) * T + kt * 128:
                                (4 + p) * T + (kt + 1) * 128],
                        qk16_sb[(hh * 64):(hh + 1) * 64,
                                p * T + qb * 512 + off:
                                p * T + (qb + 1) * 512],
                        start=True, stop=True,
                        tile_position=(hh * 64, 0))
                sc_tiles[(p, qb, kt)] = (sc, off)

            def pv_mm(p, qb, kt, hh, pv_t, nkt):
                et, off = et_tiles[(p, qb, kt)]
                nc.tensor.matmul(
                    pv_t[:, off:512],
                    v_sb[:, kt * VSTR + (2 * p + hh) * 65:
                         kt * VSTR + (2 * p + hh) * 65 + 65],
                    et[:, hh * 512 + off: (hh + 1) * 512],
                    start=(kt == 0), stop=(kt == nkt - 1))
                if hh == 1:
                    del et_tiles[(p, qb, kt)]

            # filler queue: heap of (deadline, seqno, cost_ns, fn)
            heap = []
            seqno = [0]

            def push(dl, cost, fn):
                heapq.heappush(heap, (dl, seqno[0], cost, fn))
                seqno[0] += 1

            GR = 860.0  # granule cost ns (4 x 512-col MMs)
            entries = []
            for p in range(NPAIR):
                for nb in range(4):
                    mq, mk = p, 4 + p
                    if p > 0:
                        dlq = _tidx(p, nb, 0) - 1
                        entries.append([dlq, GR,
                                        (lambda m=mq, n=nb: qk_half(m, n, 0))])
                        entries.append([dlq, GR,
                                        (lambda m=mq, n=nb: qk_half(m, n, 1))])
                        dlk = _tidx(p, nb, 4 * nb) - 1
                        entries.append([dlk, GR,
                                        (lambda m=mk, n=nb: qk_half(m, n, 0))])
                        entries.append([dlk, GR,
                                        (lambda m=mk, n=nb: qk_half(m, n, 1))])
            for tt in range(12, NTT):
                dlv = _tidx(0, tt // 4, tt)
                entries.append([dlv, GR, (lambda t=tt: v_half(t, 0))])
                entries.append([dlv, GR, (lambda t=tt: v_half(t, 1))])
            # backward-pass spread: slots >=2 tiles apart, never past deadline
            entries.sort(key=lambda e: e[0])
            slot = 10 ** 6
            for e in reversed(entries):
                slot = min(e[0], slot - 2)
                e[0] = slot
            for e in entries:
                e[0] = max(e[0], 1)
            for dl, cost, fn in entries:
                push(dl, cost, fn)

            # ---- head phase ----
            qk_chunk(0, 0)
            qk_chunk(4, 0)
            v_chunk(0)
            v_chunk(1)
            qk_chunk(0, 1)
            v_chunk(2)
            v_chunk(3)
            qk_chunk(4, 1)
            v_chunk(4)
            v_chunk(5)

            debt = [0.0]

            def pump(i):
                pj_popped = False
                while heap and heap[0][0] <= i:
                    _, _, cost, fn = heapq.heappop(heap)
                    fn()
                    debt[0] -= cost
                    pj_popped = pj_popped or cost >= 500.0
                while heap and debt[0] >= heap[0][2]:
                    if heap[0][2] >= 500.0:
                        if pj_popped:
                            break
                        pj_popped = True
                    _, _, cost, fn = heapq.heappop(heap)
                    fn()
                    debt[0] -= cost

            emit_s(*seq[0])
            for i, (p, qb, kt) in enumerate(seq):
                nkt = 4 * qb + 4
                off = max(kt - 4 * qb, 0) * 128
                w = 512 - off
                pump(i)
                if i + 1 < len(seq):
                    emit_s(*seq[i + 1])
                if kt == 0:
                    pv_h0 = pp_pv.tile([65, 512], f32, tag="pv", name="pvh0")
                sc, off2 = sc_tiles.pop((p, qb, kt))
                et = epool.tile([128, 1024], bf16, tag="exp")
                et3 = et[:].rearrange("p (h c) -> p h c", h=2)
                sc3 = sc[:].rearrange("p (h c) -> p h c", h=2)
                nc.scalar.activation(
                    et3[:, :, off:512], sc3[:, :, off:512],
                    AF.Exp, scale=float(SCALE))
                if kt - 4 * qb >= 0:
                    nc.vector.tensor_mul(
                        et3[:, :, off:off + 128],
                        et3[:, :, off:off + 128],
                        mask_sb[:].rearrange("p (h c) -> p h c", h=2))
                et_tiles[(p, qb, kt)] = (et, off)
                pv_mm(p, qb, kt, 0, pv_h0, nkt)
                # head-1 PV trails by 2 tiles (keeps PE dense, ring-3 psum)
                if kt == 2:
                    pv_h1 = pp_pv.tile([65, 512], f32, tag="pv", name="pvh1")
                    pv_mm(p, qb, 0, 1, pv_h1, nkt)
                elif kt > 2:
                    pv_mm(p, qb, kt - 2, 1, pv_h1, nkt)
                debt[0] += (2 * w + 180) / 1.2 - (3 * w / 2.4 + 90)
                debt[0] = min(debt[0], 1000.0)
                if kt == nkt - 1:
                    norm(p, qb, 0, pv_h0)
                    pv_mm(p, qb, nkt - 2, 1, pv_h1, nkt)
                    pv_mm(p, qb, nkt - 1, 1, pv_h1, nkt)
                    norm(p, qb, 1, pv_h1)

            # ---- drain remaining filler ----
            while heap:
                _, _, _, fn = heapq.heappop(heap)
                fn()
            # ---- dense out-projection tail (big pool, full rate) ----
            for tb2 in range(2):
                for md in range(8):
                    out_chunk(md, tb2)
            if _dbg:
                nc.sync.dma_start(dbg_qk_d, qk16_sb[:])
                nc.sync.dma_start(dbg_v_d, v_sb[:])
                nc.sync.dma_start(dbg_at_d, attn_sb[:])

    nc.compile()
    return nc


def _get_program():
    if "nc" not in _CACHE:
        _CACHE["nc"] = _build_program()
    return _CACHE["nc"]


def _make_core_inputs(x, w_qkv, b_qkv, w_out):
    bf = ml_dtypes.bfloat16
    mask = np.triu(np.ones((128, 128), np.float32))
    mask2 = np.concatenate([mask, mask], axis=1).astype(bf)
    ins = []
    for c in range(8):
        b, hg = c // 2, c % 2
        h0 = hg * 512
        qsel = slice(h0, h0 + 512)
        ksel = slice(D + h0, D + h0 + 512)
        vsel = slice(2 * D + h0, 2 * D + h0 + 512)
        xT = np.ascontiguousarray(x[b].T)                       # [1024, T]
        x16 = xT.astype(bf) \
            .reshape(KT, 128, T).transpose(1, 0, 2).reshape(128, KT * T)
        wqk = np.concatenate([w_qkv[qsel], w_qkv[ksel]], axis=0)  # [1024, D]
        wqk16 = wqk.astype(bf) \
            .reshape(8, 128, 8, 128).transpose(3, 0, 2, 1).reshape(128, 8192)
        wv16 = np.ascontiguousarray(w_qkv[vsel].T).astype(bf) \
            .reshape(KT, 128, 512).transpose(1, 0, 2).reshape(128, KT * 512)
        bqk = np.concatenate([b_qkv[qsel], b_qkv[ksel]])
        ins.append({
            "x16": np.ascontiguousarray(x16),
            "wqk16": np.ascontiguousarray(wqk16),
            "wv16": np.ascontiguousarray(wv16),
            "bqk": np.ascontiguousarray(bqk.reshape(8, 128).T.astype(np.float32)),
            "bv": np.ascontiguousarray(
                np.broadcast_to(b_qkv[vsel], (128, 512)).astype(np.float32)),
            "woT": np.ascontiguousarray(
                w_out[:, h0:h0 + 512].T).astype(bf),
            "mask2": mask2,
        })
    return ins


def kernel(x, w_qkv, b_qkv, w_out, b_out, _trace=False):
    from concourse.bass_utils import run_bass_kernel_spmd

    x = np.asarray(x, np.float32)
    w_qkv = np.asarray(w_qkv, np.float32)
    b_qkv = np.asarray(b_qkv, np.float32)
    w_out = np.asarray(w_out, np.float32)
    b_out = np.asarray(b_out, np.float32)

    nc = _get_program()
    ins = _make_core_inputs(x, w_qkv, b_qkv, w_out)
    res = run_bass_kernel_spmd(nc, ins, core_ids=list(range(8)), trace=_trace)
    _CACHE["last_result"] = res

    out = np.empty((B, T, D), np.float32)
    for b in range(B):
        s = res.results[2 * b]["outT"].astype(np.float32) \
            + res.results[2 * b + 1]["outT"].astype(np.float32)
        out[b] = s.T + b_out
    return out
